# revision 1
# baseline (speedup 1.0000x reference)
"""Trainium2 Bass kernel for nn_Dependence_Learning (sparse_attention).

Computation (faithful to the reference's row-major reshapes):
  tri1 = relu(BN(x @ w1))   # key map
  tri2 = relu(BN(x @ w2))   # query map
  Flat t-major space per (b, ch): F[t*4096 + l] = shifted image (9 shifts of
  the k=3 window, zero padded).  The reference's scrambled reshapes make the
  softmax run over consecutive GROUPS OF 9 of this flat array, with the
  "center" at group offset 4:
    lg[p] = F1[p]*F2[9*(p//9)+4] + F1[9*(p//9)+4]*F2[p]
    SM    = softmax over each group of 9
    pre[l] = sum_t SM[t*4096+l] * Fx[t*4096+l]
  out = relu(BN(pre @ wf))

Sharding: each of the 8 cores owns 32 of the 256 channels for conv1/conv2 and
the attention stage (BN stats are per-channel -> fully local), then an
AllGather of `pre` lets each core compute 32 output channels of the final
conv (again with local BN stats).

Per-core pair layout: partition p = 32*b + j holds (batch b, local channel j).

The attention stage runs over 4 column quarters x 9 shift segments.  For
quarter q / segment t the native flat range [4096 t + 1024 q, +1024) is
extended both ways to group-of-9 boundaries; every group is complete inside
the extent and only the native range is accumulated.  Each quarter's `pre`
columns are AllGathered as soon as they finish, overlapping the remaining
attention compute, and the final conv consumes the gathered quarters as they
arrive.

Matmuls run in float32r (~1.6e-4 rel err, 4x fp32 throughput).
"""

import numpy as np
from contextlib import ExitStack

import concourse.bass as bass
import concourse.bacc as bacc
import concourse.tile as tile
import concourse.mybir as mybir
from concourse.bass_utils import run_bass_kernel_spmd

F32 = mybir.dt.float32
F32R = mybir.dt.float32r
AF = mybir.ActivationFunctionType
AX = mybir.AxisListType

NCORES = 8
B = 4
C = 256
HH = 64
L = HH * HH           # 4096
CS = C // NCORES      # 32 channels per core
P = 128               # partitions; == B * CS
PH = HH + 2           # 66 padded
PSZ = PH * PH         # 4356
NQ = 4                # column quarters
QW = L // NQ          # 1024
QR = QW // HH         # 16 rows per quarter
# uneven column chunks: big ones amortize, small ones shrink the exit tail
CHUNKS = [(0, 1024), (1024, 1024), (2048, 1024), (3072, 768), (3840, 256)]
NCH = len(CHUNKS)
NLC = 8               # conv l chunks
LCW = L // NLC        # 512
NSTAT = B * L         # 16384 samples per channel for BN
EPS = 1e-5
EXPB = -30.0          # constant exp bias (softmax shift; ratios unchanged)
EXTMAX = max(w for _, w in CHUNKS) + 16      # >= 1024 + r + s

assert L % 9 == 1


def _shift(t):
    return t // 3 - 1, t % 3 - 1


def build_program():
    nc = bacc.Bacc("TRN2", target_bir_lowering=False, num_devices=NCORES)

    x_t = nc.dram_tensor("x", [B, C, L], F32R, kind="ExternalInput")
    xs_t = nc.dram_tensor("xs", [P, L], F32, kind="ExternalInput")
    w1t_t = nc.dram_tensor("w1t", [C, CS], F32R, kind="ExternalInput")
    w2t_t = nc.dram_tensor("w2t", [C, CS], F32R, kind="ExternalInput")
    wfl_t = nc.dram_tensor("wfl", [2, CS, P], F32R, kind="ExternalInput")
    bnp_t = nc.dram_tensor("bnp", [6, CS], F32, kind="ExternalInput")
    out_t = nc.dram_tensor("out", [P, L], F32, kind="ExternalOutput")

    with tile.TileContext(nc) as tc, ExitStack() as top:
        consts = top.enter_context(tc.tile_pool(name="consts", bufs=1))
        persist = top.enter_context(tc.tile_pool(name="persist", bufs=1))
        tiny = top.enter_context(tc.tile_pool(name="tiny", bufs=4))
        dram = top.enter_context(tc.tile_pool(name="dram", bufs=1, space="DRAM"))

        # ---- weights / constants ----
        # conv weights, zero-padded so batch b's 32 out channels sit at
        # column band 32b (f32r matmul cannot column-tile the dst)
        wt = {}
        for name, t in (("w1", w1t_t), ("w2", w2t_t)):
            for h in range(2):
                for b in range(B):
                    w = consts.tile([P, P], F32R, tag=f"{name}p{h}{b}",
                                    name=f"{name}p{h}{b}")
                    nc.vector.memset(w[:, :].bitcast(F32), 0.0)
                    nc.sync.dma_start(out=w[:, CS * b:CS * (b + 1)],
                                      in_=t[h * P:(h + 1) * P, :])
                    wt[(name, h, b)] = w
        # wf for the partial final conv: [32, 128] block replicated on all
        # four 32-partition bands (lhsT rows must sit on the rhs partitions)
        wfl = []
        for h in range(2):
            wr = consts.tile([P, P], F32R, tag=f"wflh{h}", name=f"wflh{h}")
            nc.gpsimd.dma_start(
                out=wr[:, :],
                in_=bass.AP(tensor=wfl_t, offset=h * CS * P,
                            ap=[[0, B], [P, CS], [1, P]]))
            wfl.append(wr)
        eps_c = consts.tile([P, 1], F32, tag="eps_c")
        nc.vector.memset(eps_c[:, :], EPS)
        expb_c = consts.tile([P, 1], F32, tag="expb_c")
        nc.vector.memset(expb_c[:, :], EXPB)
        # selector for cross-b stat combine: sel[p, j] = 1 if p % 32 == j
        sel_t = nc.inline_tensor(
            np.tile(np.eye(CS, dtype=np.float32), (B, 1)), name="sel")
        sel = consts.tile([P, CS], F32, tag="sel")
        nc.gpsimd.dma_start(out=sel[:, :], in_=sel_t[:, :])
        id_t = nc.inline_tensor(np.eye(P, dtype=np.float32), name="idm")
        idm = consts.tile([P, P], F32R, tag="idm")
        nc.gpsimd.dma_start(out=idm[:, :], in_=id_t[:, :].bitcast(F32R))
        selT_t = nc.inline_tensor(
            np.tile(np.eye(CS, dtype=np.float32), (1, B)), name="selT")
        selT = consts.tile([CS, P], F32, tag="selT")
        nc.gpsimd.dma_start(out=selT[:, :], in_=selT_t[:, :])
        # BN params as per-partition columns [32, 1] (g_i, be_i for i=0,1,2)
        bncol = []
        for i in range(6):
            col = consts.tile([CS, 1], F32, tag=f"bncol{i}", name=f"bncol{i}")
            nc.sync.dma_start(
                out=col[:, :],
                in_=bass.AP(tensor=bnp_t, offset=i * CS, ap=[[1, CS], [0, 1]]))
            bncol.append(col)

        ab = [persist.tile([P, 2], F32, tag=f"ab{i}", name=f"ab{i}")
              for i in range(3)]
        sums = [persist.tile([P, NLC], F32, tag=f"sums{i}", name=f"sums{i}")
                for i in range(4)]
        sumsf = [persist.tile([P, NCH], F32, tag=f"sumsf{i}", name=f"sumsf{i}")
                 for i in range(2)]

        partd = [dram.tile([NCORES * P, w], F32, tag=f"partd{ci}",
                           name=f"partd{ci}")
                 for ci, (c0, w) in enumerate(CHUNKS)]
        recvd = [dram.tile([P, w], F32, tag=f"recvd{ci}", name=f"recvd{ci}")
                 for ci, (c0, w) in enumerate(CHUNKS)]

        def bn_coeffs(sum_ap, sumsq_ap, pidx, ab_out, n):
            """Per-channel [32,1] stats -> ab_out[:, 0:1] = g/sqrt(var+eps),
            ab_out[:, 1:2] = be - mean*a.  pidx picks the (g, be) pair."""
            mean = tiny.tile([CS, 1], F32, tag="mean")
            var = tiny.tile([CS, 1], F32, tag="var")
            sd = tiny.tile([CS, 1], F32, tag="sd")
            tmp = tiny.tile([CS, 1], F32, tag="tmp")
            nc.vector.tensor_scalar_mul(mean[:, :], sum_ap, 1.0 / n)
            nc.vector.tensor_scalar_mul(var[:, :], sumsq_ap, 1.0 / n)
            nc.vector.tensor_mul(tmp[:, :], mean[:, :], mean[:, :])
            nc.vector.tensor_sub(var[:, :], var[:, :], tmp[:, :])
            nc.scalar.activation(sd[:, :], var[:, :], AF.Sqrt,
                                 bias=eps_c[0:CS, :])
            nc.vector.reciprocal(sd[:, :], sd[:, :])
            nc.vector.tensor_mul(ab_out[:, 0:1], bncol[2 * pidx][:, :], sd[:, :])
            nc.vector.tensor_mul(tmp[:, :], mean[:, :], ab_out[:, 0:1])
            nc.vector.tensor_sub(ab_out[:, 1:2], bncol[2 * pidx + 1][:, :],
                                 tmp[:, :])

        with tc.tile_pool(name="imgs", bufs=1) as imgs:
            img1 = imgs.tile([P, PSZ], F32, tag="img1")
            img2 = imgs.tile([P, PSZ], F32, tag="img2")
            img3 = imgs.tile([P, PSZ], F32, tag="img3")
            i3d = {}
            for nm, im in (("1", img1), ("2", img2), ("3", img3)):
                v = im[:, :].rearrange("p (r c) -> p r c", c=PH)
                i3d[nm] = v
                nc.vector.memset(v[:, 0, :], 0.0)
                nc.vector.memset(v[:, PH - 1, :], 0.0)
                nc.vector.memset(v[:, 1:PH - 1, 0], 0.0)
                nc.vector.memset(v[:, 1:PH - 1, PH - 1], 0.0)

            # x pair-layout load into img3 interior -- emitted on the gpsimd
            # (SWDGE) queue so it never delays the conv rhs stream on sync
            nc.gpsimd.dma_start(
                out=i3d["3"][:, 1:1 + HH, 1:1 + HH],
                in_=bass.AP(tensor=xs_t, offset=0,
                            ap=[[L, P], [HH, HH], [1, HH]]),
            )

            # ================= phase 1: conv1 + conv2 + BN stats =========
            with ExitStack() as s1:
                rhsp = s1.enter_context(tc.tile_pool(name="rhs", bufs=6))
                psump = s1.enter_context(
                    tc.tile_pool(name="psum", bufs=2, space="PSUM"))
                evict = s1.enter_context(tc.tile_pool(name="evict", bufs=2))
                ybuf = s1.enter_context(tc.tile_pool(name="ybuf", bufs=1))
                y1s = ybuf.tile([P, L], F32, tag="y1")
                y2s = ybuf.tile([P, L], F32, tag="y2")

                for lc in range(NLC):
                    p1 = psump.tile([P, LCW], F32, tag="p1")
                    p2 = psump.tile([P, LCW], F32, tag="p2")
                    for b in range(B):
                        rt = []
                        for h in range(2):
                            r = rhsp.tile([P, LCW], F32R, tag="rhs")
                            nc.sync.dma_start(
                                out=r[:, :],
                                in_=bass.AP(
                                    tensor=x_t,
                                    offset=b * C * L + h * P * L + lc * LCW,
                                    ap=[[L, P], [1, LCW]]),
                            )
                            rt.append(r)
                        for h in range(2):
                            nc.tensor.matmul(
                                p1[:, :], wt[("w1", h, b)][:, :],
                                rt[h][:, :],
                                start=(b == 0 and h == 0),
                                stop=(b == B - 1 and h == 1),
                                tile_position=(0, 0))
                        for h in range(2):
                            nc.tensor.matmul(
                                p2[:, :], wt[("w2", h, b)][:, :],
                                rt[h][:, :],
                                start=(b == 0 and h == 0),
                                stop=(b == B - 1 and h == 1),
                                tile_position=(0, 0))
                    for (pp, ys, si) in ((p1, y1s, 0), (p2, y2s, 2)):
                        nc.scalar.activation(
                            out=ys[:, lc * LCW:(lc + 1) * LCW], in_=pp[:, :],
                            func=AF.Copy, accum_out=sums[si][:, lc:lc + 1])
                        sq = evict.tile([P, LCW], F32, tag="sq")
                        nc.scalar.activation(
                            out=sq[:, :], in_=pp[:, :],
                            func=AF.Square, accum_out=sums[si + 1][:, lc:lc + 1])

                # ---- stats: fold chunks, combine across b via PE selector --
                stats4 = persist.tile([P, 4], F32, tag="stats4")
                for i in range(4):
                    nc.vector.reduce_sum(out=stats4[:, i:i + 1],
                                         in_=sums[i][:, :], axis=AX.X)
                stp = psump.tile([CS, 4], F32, tag="stp")
                nc.tensor.matmul(stp[:, :], sel[:, :], stats4[:, :],
                                 start=True, stop=True, tile_position=(0, 0))
                statjs = persist.tile([CS, 4], F32, tag="statjs")
                nc.scalar.activation(out=statjs[:, :], in_=stp[:, :],
                                     func=AF.Copy)
                abj = persist.tile([CS, 4], F32, tag="abj")
                for i in range(2):
                    bn_coeffs(statjs[:, 2 * i:2 * i + 1],
                              statjs[:, 2 * i + 1:2 * i + 2], i,
                              abj[:, 2 * i:2 * i + 2], NSTAT)
                ab_d = dram.tile([CS, 4], F32, tag="ab_d")
                nc.sync.dma_start(out=ab_d[:, :], in_=abj[:, :])
                for i in range(2):
                    nc.sync.dma_start(
                        out=ab[i][:, :],
                        in_=bass.AP(tensor=ab_d.tensor,
                                    offset=ab_d.offset + 2 * i,
                                    ap=[[0, B], [4, CS], [1, 2]]))

                # ---- BN apply + relu into padded interiors ----
                for (ys, im, i) in ((y1s, i3d["1"], 0), (y2s, i3d["2"], 1)):
                    nc.scalar.activation(
                        out=im[:, 1:1 + HH, 1:1 + HH],
                        in_=ys[:, :].rearrange("p (r c) -> p r c", c=HH),
                        func=AF.Relu,
                        bias=ab[i][:, 1:2], scale=ab[i][:, 0:1])

            # ===== phase 2+3: attention quarters, gathers, final conv =====
            with ExitStack() as s2:
                fpool = s2.enter_context(tc.tile_pool(name="fbuf", bufs=2))
                tpool = s2.enter_context(tc.tile_pool(name="tbuf", bufs=2))
                gpool = s2.enter_context(tc.tile_pool(name="gbuf", bufs=2))
                orpool = s2.enter_context(tc.tile_pool(name="outr", bufs=2))
                psumf = s2.enter_context(
                    tc.tile_pool(name="psumf", bufs=2, space="PSUM"))
                evictf = s2.enter_context(tc.tile_pool(name="evictf", bufs=2))
                ybuff = s2.enter_context(tc.tile_pool(name="ybuff", bufs=1))
                yfs = ybuff.tile([P, L], F32, tag="yf")
                fout = ybuff.tile([P, L], F32, tag="fout")

                OUTr_of = {}

                def sub_widths(w):
                    return [(i * LCW, min(LCW, w - i * LCW))
                            for i in range((w + LCW - 1) // LCW)]

                def partial_piece(ci, b, h, off, ww):
                    c0, w = CHUNKS[ci]
                    OUTr = OUTr_of[ci]
                    pf = psumf.tile([P, LCW], F32, tag="pf", name="pf")
                    nc.tensor.matmul(
                        pf[:, :ww],
                        wfl[h][CS * b:CS * (b + 1), :],
                        OUTr[CS * b:CS * (b + 1), off:off + ww],
                        start=True, stop=True,
                        tile_position=(CS * b, 0))
                    ev = evictf.tile([P, LCW], F32, tag="ev", name="ev")
                    nc.scalar.activation(out=ev[:, :ww], in_=pf[:, :ww],
                                         func=AF.Copy)
                    # partd addr(k=4h+a, p'=32b+j, l'); psum partition 32a+j
                    nc.sync.dma_start(
                        out=bass.AP(
                            tensor=partd[ci].tensor,
                            offset=(partd[ci].offset + h * 4 * P * w
                                    + b * CS * w + off),
                            ap=[[P * w, 4], [w, CS], [1, ww]]),
                        in_=ev[:, :ww])

                def emit_rs_cc(ci):
                    nc.gpsimd.collective_compute(
                        "ReduceScatter", mybir.AluOpType.add,
                        replica_groups=[list(range(NCORES))],
                        ins=[partd[ci][:, :].opt()],
                        outs=[recvd[ci][:, :].opt()])

                def emit_rs_post(ci):
                    c0, w = CHUNKS[ci]
                    nc.sync.dma_start(out=yfs[:, c0:c0 + w],
                                      in_=recvd[ci][:, :])
                    sq = evictf.tile([P, QW], F32, tag="sqf", name="sqf")
                    nc.scalar.activation(
                        out=sq[:, :w], in_=yfs[:, c0:c0 + w],
                        func=AF.Copy, accum_out=sumsf[0][:, ci:ci + 1])
                    nc.scalar.activation(
                        out=sq[:, :w], in_=yfs[:, c0:c0 + w],
                        func=AF.Square, accum_out=sumsf[1][:, ci:ci + 1])

                def pieces_of(ci):
                    return [(b, h, off, ww) for h in range(2)
                            for (off, ww) in sub_widths(CHUNKS[ci][1])
                            for b in range(B)]

                def emit_A(ci, t):
                    """one (chunk, shift-segment) iteration of the attention stage."""
                    c0, w = CHUNKS[ci]
                    row0, nrows = c0 // HH, w // HH
                    di, dj = _shift(t)
                    r = (t + c0) % 9
                    s = (-(t + c0 + w)) % 9
                    ext = w + r + s
                    ng = ext // 9
                    F1c = fpool.tile([P, EXTMAX], F32, tag="F1c", name="F1c")
                    F2c = fpool.tile([P, EXTMAX], F32, tag="F2c", name="F2c")
                    for (Fc, im, iflat) in ((F1c, i3d["1"], img1),
                                            (F2c, i3d["2"], img2)):
                        nc.scalar.copy(
                            out=Fc[:, r:r + w].rearrange(
                                "p (i j) -> p i j", j=HH),
                            in_=im[:, 1 + di + row0:1 + di + row0 + nrows,
                                   1 + dj:1 + dj + HH])
                        if r:
                            if c0:
                                off = (di + row0) * PH + (65 - r + dj)
                            else:
                                pdi, pdj = _shift(t - 1)
                                off = (64 + pdi) * PH + (65 - r + pdj)
                            nc.sync.dma_start(out=Fc[:, 0:r],
                                              in_=iflat[:, off:off + r])
                        if s:
                            if c0 + w < L:
                                off = (1 + di + row0 + nrows) * PH + 1 + dj
                            else:
                                ndi, ndj = _shift(t + 1)
                                off = (1 + ndi) * PH + (1 + ndj)
                            nc.sync.dma_start(out=Fc[:, r + w:r + w + s],
                                              in_=iflat[:, off:off + s])

                    F1g = F1c[:, :9 * ng].rearrange("p (g s) -> p g s", s=9)
                    F2g = F2c[:, :9 * ng].rearrange("p (g s) -> p g s", s=9)
                    # centers read straight from the F tiles
                    cqb = F2g[:, :, 4].unsqueeze(2).broadcast_to((P, ng, 9))
                    ckb = F1g[:, :, 4].unsqueeze(2).broadcast_to((P, ng, 9))

                    t1 = tpool.tile([P, EXTMAX], F32, tag="t1", name="t1")
                    t2 = tpool.tile([P, EXTMAX], F32, tag="t2", name="t2")
                    lgb = tpool.tile([P, EXTMAX], F32, tag="lgb", name="lgb")
                    ext = w + r + s
                    t1g = t1[:, :9 * ng].rearrange("p (g s) -> p g s", s=9)
                    t2g = t2[:, :9 * ng].rearrange("p (g s) -> p g s", s=9)
                    nc.vector.tensor_mul(t1g, F1g, cqb)
                    nc.vector.tensor_mul(t2g, F2g, ckb)
                    nc.vector.tensor_add(lgb[:, :ext], t1[:, :ext],
                                         t2[:, :ext])
                    nc.scalar.activation(out=t1[:, :ext], in_=lgb[:, :ext],
                                         func=AF.Exp, bias=expb_c[:, :])
                    S = gpool.tile([P, 116], F32, tag="S", name="S")
                    R = gpool.tile([P, 116], F32, tag="R", name="R")
                    nc.vector.reduce_sum(out=S[:, :ng], in_=t1g, axis=AX.X)
                    nc.vector.reciprocal(R[:, :ng], S[:, :ng])
                    nc.vector.tensor_mul(
                        t2g, t1g,
                        R[:, :ng].unsqueeze(2).broadcast_to((P, ng, 9)))
                    Pt = tpool.tile([P, QW], F32R, tag="Pt", name="Pt")
                    nc.vector.tensor_mul(
                        Pt[:, :w].rearrange("p (i j) -> p i j", j=HH),
                        t2[:, r:r + w].rearrange("p (i j) -> p i j", j=HH),
                        i3d["3"][:, 1 + di + row0:1 + di + row0 + nrows,
                                 1 + dj:1 + dj + HH])
                    OUTP = OUTq_cur[0]
                    for (off, ww) in sub_widths(w):
                        nc.tensor.matmul(
                            OUTP[:, off:off + ww],
                            idm[:, :],
                            Pt[:, off:off + ww],
                            start=(t == 0), stop=(t == 8),
                            tile_position=(0, 0), skip_group_check=True)

                OUTq_cur = [None]
                for ci, (c0, w) in enumerate(CHUNKS):
                    prev = pieces_of(ci - 1) if ci > 0 else []
                    OUTP = psumf.tile([P, QW], F32, tag="OUTP",
                                      name=f"OUTP{ci}")
                    OUTq_cur[0] = OUTP
                    for t in range(9):
                        # interleave previous chunk's partial final conv
                        if ci > 0 and t < 8:
                            for pc in prev[2 * t:2 * t + 2]:
                                partial_piece(ci - 1, *pc)
                        elif ci > 0 and t == 8:
                            emit_rs_cc(ci - 1)
                        if ci > 1 and t == 2:
                            emit_rs_post(ci - 2)
                        emit_A(ci, t)

                    # round pre chunk to f32r for the partial conv PE pass
                    OUTr = orpool.tile([P, QW], F32R, tag="OUTr",
                                       name="OUTr")
                    nc.vector.tensor_copy(out=OUTr[:, :w], in_=OUTP[:, :w])
                    OUTr_of[ci] = OUTr

                # drain the last chunk's partial conv + RS
                for pc in pieces_of(NCH - 1):
                    partial_piece(NCH - 1, *pc)
                emit_rs_cc(NCH - 1)
                emit_rs_post(NCH - 2)
                emit_rs_post(NCH - 1)

                # ---- final BN stats + apply + output ----
                stats2 = persist.tile([P, 2], F32, tag="stats2")
                for i in range(2):
                    nc.vector.reduce_sum(out=stats2[:, i:i + 1],
                                         in_=sumsf[i][:, :], axis=AX.X)
                stpf = psumf.tile([CS, 2], F32, tag="stpf", bufs=1)
                nc.tensor.matmul(stpf[:, :], sel[:, :], stats2[:, :],
                                 start=True, stop=True, tile_position=(0, 0))
                statjsf = persist.tile([CS, 2], F32, tag="statjsf")
                nc.scalar.activation(out=statjsf[:, :], in_=stpf[:, :],
                                     func=AF.Copy)
                abjf = persist.tile([CS, 2], F32, tag="abjf")
                bn_coeffs(statjsf[:, 0:1], statjsf[:, 1:2], 2,
                          abjf[:, 0:2], NSTAT)
                abpf = psumf.tile([P, 2], F32, tag="abpf", bufs=1)
                nc.tensor.matmul(abpf[:, :], selT[:, :], abjf[:, :],
                                 start=True, stop=True, tile_position=(0, 0))
                nc.scalar.activation(out=ab[2][:, :], in_=abpf[:, :],
                                     func=AF.Copy)
                nc.scalar.activation(out=fout[:, :], in_=yfs[:, :],
                                     func=AF.Relu,
                                     bias=ab[2][:, 1:2], scale=ab[2][:, 0:1])
                nc.sync.dma_start(out=out_t[:, :], in_=fout[:, :])

    nc.finalize()
    return nc


_NC_CACHE = None


def _get_nc():
    global _NC_CACHE
    if _NC_CACHE is None:
        _NC_CACHE = build_program()
    return _NC_CACHE


def make_in_maps(inputs):
    x = np.ascontiguousarray(np.asarray(inputs["x"], np.float32).reshape(B, C, L))
    maps = []
    for k in range(NCORES):
        sl = slice(k * CS, (k + 1) * CS)
        m = {
            "x": x,
            "xs": np.ascontiguousarray(x[:, sl, :].reshape(P, L)),
            "w1t": np.ascontiguousarray(np.asarray(inputs["w1"], np.float32)[sl, :].T),
            "w2t": np.ascontiguousarray(np.asarray(inputs["w2"], np.float32)[sl, :].T),
            "wfl": np.ascontiguousarray(np.stack([
                np.asarray(inputs["wf"], np.float32)[h * P:(h + 1) * P, sl].T
                for h in range(2)])),
            "bnp": np.ascontiguousarray(np.stack([
                np.asarray(inputs["g1"], np.float32)[sl],
                np.asarray(inputs["be1"], np.float32)[sl],
                np.asarray(inputs["g2"], np.float32)[sl],
                np.asarray(inputs["be2"], np.float32)[sl],
                np.asarray(inputs["gf"], np.float32)[sl],
                np.asarray(inputs["bef"], np.float32)[sl],
            ])),
        }
        maps.append(m)
    return maps


def run(inputs, trace=False):
    nc = _get_nc()
    in_maps = make_in_maps(inputs)
    res = run_bass_kernel_spmd(nc, in_maps, core_ids=list(range(NCORES)),
                               trace=trace)
    full = np.empty((B, C, HH, HH), np.float32)
    for k in range(NCORES):
        full[:, k * CS:(k + 1) * CS] = res.results[k]["out"].reshape(B, CS, HH, HH)
    return full, res


def kernel(**inputs) -> np.ndarray:
    out, _ = run(inputs, trace=False)
    return out



# revision 8
# speedup vs baseline: 1.2788x; 1.2788x over previous
"""Trainium2 Bass kernel for nn_Dependence_Learning (sparse_attention).

Computation (faithful to the reference's row-major reshapes):
  tri1 = relu(BN(x @ w1))   # key map
  tri2 = relu(BN(x @ w2))   # query map
  Flat t-major space per (b, ch): F[t*4096 + l] = shifted image (9 shifts of
  the k=3 window, zero padded).  The reference's scrambled reshapes make the
  softmax run over consecutive GROUPS OF 9 of this flat array, with the
  "center" at group offset 4:
    lg[p] = F1[p]*F2[9*(p//9)+4] + F1[9*(p//9)+4]*F2[p]
    SM    = softmax over each group of 9
    pre[l] = sum_t SM[t*4096+l] * Fx[t*4096+l]
  out = relu(BN(pre @ wf))

Sharding: each of the 8 cores owns 32 of the 256 channels for conv1/conv2 and
the attention stage (BN stats are per-channel -> fully local), then a bf16
ReduceScatter of the partial final conv gives each core its 32 output
channels (again with local BN stats).

Per-core pair layout: partition p = 32*b + j holds (batch b, local channel j).

The whole datapath runs in bf16 (DVE 2x packed mode, halved DMA + collective
bytes); only BN statistics, softmax denominators and PSUM accumulation stay
fp32.  Attention work is spread over three engines:
  DVE : window extract F1, logits muls/add (packed), group-sum tree, recip,
        normalize, x-multiply
  ACT : exp, center/recip broadcast materialization, psum evictions, BN
  Pool: window extract F2, center broadcast C1p
"""

import numpy as np
import ml_dtypes
from contextlib import ExitStack

import concourse.bass as bass
import concourse.bacc as bacc
import concourse.tile as tile
import concourse.mybir as mybir
from concourse.bass_utils import run_bass_kernel_spmd

F32 = mybir.dt.float32
BF = mybir.dt.bfloat16
AF = mybir.ActivationFunctionType
AX = mybir.AxisListType

NCORES = 8
B = 4
C = 256
HH = 64
L = HH * HH           # 4096
CS = C // NCORES      # 32 channels per core
P = 128               # partitions; == B * CS
PH = HH + 2           # 66 padded
PSZ = PH * PH         # 4356
NLC = 8               # conv l chunks
LCW = L // NLC        # 512
NSTAT = B * L         # 16384 samples per channel for BN
EPS = 1e-5
EXPB = -20.0          # constant exp bias (softmax shift; ratios unchanged)
# attention chunks: row-aligned, <=1536 so psum fits 3 banks
CHUNKS = [(0, 1536), (1536, 1536), (3072, 1024)]
NCH = len(CHUNKS)
EXTMAX = max(w for _, w in CHUNKS) + 16
NGMAX = EXTMAX // 9 + 1

assert L % 9 == 1


def _shift(t):
    return t // 3 - 1, t % 3 - 1


def build_program():
    nc = bacc.Bacc("TRN2", target_bir_lowering=False, num_devices=NCORES)

    xb_t = nc.dram_tensor("xb", [B, C, L], BF, kind="ExternalInput")
    xsb_t = nc.dram_tensor("xsb", [P, L], BF, kind="ExternalInput")
    w1p_t = nc.dram_tensor("w1p", [P, 8 * P], BF, kind="ExternalInput")
    w2p_t = nc.dram_tensor("w2p", [P, 8 * P], BF, kind="ExternalInput")
    wfl_t = nc.dram_tensor("wfl", [P, 2 * P], BF, kind="ExternalInput")
    bnp_t = nc.dram_tensor("bnp", [6, CS], F32, kind="ExternalInput")
    out_t = nc.dram_tensor("out", [P, L], BF, kind="ExternalOutput")

    with tile.TileContext(nc) as tc, ExitStack() as top:
        consts = top.enter_context(tc.tile_pool(name="consts", bufs=1))
        persist = top.enter_context(tc.tile_pool(name="persist", bufs=1))
        tiny = top.enter_context(tc.tile_pool(name="tiny", bufs=4))
        dram = top.enter_context(tc.tile_pool(name="dram", bufs=1, space="DRAM"))

        # ---- weights / constants ----
        w1s = consts.tile([P, 8 * P], BF, tag="w1s")
        nc.sync.dma_start(out=w1s[:, :], in_=w1p_t[:, :])
        w2s = consts.tile([P, 8 * P], BF, tag="w2s")
        nc.sync.dma_start(out=w2s[:, :], in_=w2p_t[:, :])
        wfs = consts.tile([P, 2 * P], BF, tag="wfs")
        nc.sync.dma_start(out=wfs[:, :], in_=wfl_t[:, :])

        eps_c = consts.tile([P, 1], F32, tag="eps_c")
        nc.vector.memset(eps_c[:, :], EPS)
        expb_c = consts.tile([P, 1], F32, tag="expb_c")
        nc.vector.memset(expb_c[:, :], EXPB)
        # selector for cross-b stat combine: sel[p, j] = 1 if p % 32 == j
        sel_t = nc.inline_tensor(
            np.tile(np.eye(CS, dtype=np.float32), (B, 1)), name="sel")
        sel = consts.tile([P, CS], F32, tag="sel")
        nc.gpsimd.dma_start(out=sel[:, :], in_=sel_t[:, :])
        id_t = nc.inline_tensor(np.eye(P, dtype=np.float32), name="idm")
        idf = consts.tile([P, P], F32, tag="idf")
        nc.gpsimd.dma_start(out=idf[:, :], in_=id_t[:, :])
        idm = consts.tile([P, P], BF, tag="idm")
        nc.scalar.copy(out=idm[:, :], in_=idf[:, :])
        selT_t = nc.inline_tensor(
            np.tile(np.eye(CS, dtype=np.float32), (1, B)), name="selT")
        selT = consts.tile([CS, P], F32, tag="selT")
        nc.gpsimd.dma_start(out=selT[:, :], in_=selT_t[:, :])
        # BN params as per-partition columns [32, 1] (g_i, be_i for i=0,1,2)
        bncol = []
        for i in range(6):
            col = consts.tile([CS, 1], F32, tag=f"bncol{i}", name=f"bncol{i}")
            nc.sync.dma_start(
                out=col[:, :],
                in_=bass.AP(tensor=bnp_t, offset=i * CS, ap=[[1, CS], [0, 1]]))
            bncol.append(col)

        ab = [persist.tile([P, 2], F32, tag=f"ab{i}", name=f"ab{i}")
              for i in range(3)]
        sums = [persist.tile([P, NLC], F32, tag=f"sums{i}", name=f"sums{i}")
                for i in range(4)]
        sumsf = [persist.tile([P, NCH], F32, tag=f"sumsf{i}", name=f"sumsf{i}")
                 for i in range(2)]

        partd = [dram.tile([NCORES * P, w], BF, tag=f"partd{ci}",
                           name=f"partd{ci}")
                 for ci, (c0, w) in enumerate(CHUNKS)]
        recvd = [dram.tile([P, w], BF, tag=f"recvd{ci}", name=f"recvd{ci}")
                 for ci, (c0, w) in enumerate(CHUNKS)]

        def bn_coeffs(sum_ap, sumsq_ap, pidx, ab_out, n):
            """Per-channel [32,1] stats -> ab_out[:, 0:1] = g/sqrt(var+eps),
            ab_out[:, 1:2] = be - mean*a.  pidx picks the (g, be) pair."""
            mean = tiny.tile([CS, 1], F32, tag="mean")
            var = tiny.tile([CS, 1], F32, tag="var")
            sd = tiny.tile([CS, 1], F32, tag="sd")
            tmp = tiny.tile([CS, 1], F32, tag="tmp")
            nc.vector.tensor_scalar_mul(mean[:, :], sum_ap, 1.0 / n)
            nc.vector.tensor_scalar_mul(var[:, :], sumsq_ap, 1.0 / n)
            nc.vector.tensor_mul(tmp[:, :], mean[:, :], mean[:, :])
            nc.vector.tensor_sub(var[:, :], var[:, :], tmp[:, :])
            nc.scalar.activation(sd[:, :], var[:, :], AF.Sqrt,
                                 bias=eps_c[0:CS, :])
            nc.vector.reciprocal(sd[:, :], sd[:, :])
            nc.vector.tensor_mul(ab_out[:, 0:1], bncol[2 * pidx][:, :], sd[:, :])
            nc.vector.tensor_mul(tmp[:, :], mean[:, :], ab_out[:, 0:1])
            nc.vector.tensor_sub(ab_out[:, 1:2], bncol[2 * pidx + 1][:, :],
                                 tmp[:, :])

        with tc.tile_pool(name="imgs", bufs=1) as imgs:
            img1 = imgs.tile([P, PSZ], BF, tag="img1")
            img2 = imgs.tile([P, PSZ], BF, tag="img2")
            img3 = imgs.tile([P, PSZ], BF, tag="img3")
            i3d = {}
            for nm, im in (("1", img1), ("2", img2), ("3", img3)):
                v = im[:, :].rearrange("p (r c) -> p r c", c=PH)
                i3d[nm] = v
                nc.vector.memset(v[:, 0, :], 0.0)
                nc.vector.memset(v[:, PH - 1, :], 0.0)
                nc.vector.memset(v[:, 1:PH - 1, 0], 0.0)
                nc.vector.memset(v[:, 1:PH - 1, PH - 1], 0.0)

            # x pair-layout: flat load, then on-chip 3D spread into img3
            xflat = persist.tile([P, L], BF, tag="xflat")
            nc.gpsimd.dma_start(out=xflat[:, :], in_=xsb_t[:, :])
            nc.scalar.copy(
                out=i3d["3"][:, 1:1 + HH, 1:1 + HH],
                in_=xflat[:, :].rearrange("p (r c) -> p r c", c=HH))

            # ================= phase 1: conv1 + conv2 + BN stats =========
            with ExitStack() as s1:
                rhsp = s1.enter_context(tc.tile_pool(name="rhs", bufs=6))
                psump = s1.enter_context(
                    tc.tile_pool(name="psum", bufs=2, space="PSUM"))
                evict = s1.enter_context(tc.tile_pool(name="evict", bufs=2))
                ybuf = s1.enter_context(tc.tile_pool(name="ybuf", bufs=1))
                y1s = ybuf.tile([P, L], BF, tag="y1")
                y2s = ybuf.tile([P, L], BF, tag="y2")

                for lc in range(NLC):
                    p1 = psump.tile([P, LCW], F32, tag="p1")
                    p2 = psump.tile([P, LCW], F32, tag="p2")
                    for b in range(B):
                        rt = []
                        for h in range(2):
                            r = rhsp.tile([P, LCW], BF, tag="rhs")
                            nc.sync.dma_start(
                                out=r[:, :],
                                in_=bass.AP(
                                    tensor=xb_t,
                                    offset=b * C * L + h * P * L + lc * LCW,
                                    ap=[[L, P], [1, LCW]]),
                            )
                            rt.append(r)
                        for h in range(2):
                            kb = h * B + b
                            nc.tensor.matmul(
                                p1[:, :], w1s[:, P * kb:P * (kb + 1)],
                                rt[h][:, :],
                                start=(b == 0 and h == 0),
                                stop=(b == B - 1 and h == 1),
                                tile_position=(0, 0))
                        for h in range(2):
                            kb = h * B + b
                            nc.tensor.matmul(
                                p2[:, :], w2s[:, P * kb:P * (kb + 1)],
                                rt[h][:, :],
                                start=(b == 0 and h == 0),
                                stop=(b == B - 1 and h == 1),
                                tile_position=(0, 0))
                    for (pp, ys, si) in ((p1, y1s, 0), (p2, y2s, 2)):
                        nc.scalar.activation(
                            out=ys[:, lc * LCW:(lc + 1) * LCW], in_=pp[:, :],
                            func=AF.Copy, accum_out=sums[si][:, lc:lc + 1])
                        sq = evict.tile([P, LCW], BF, tag="sq")
                        nc.scalar.activation(
                            out=sq[:, :], in_=pp[:, :],
                            func=AF.Square, accum_out=sums[si + 1][:, lc:lc + 1])

                # ---- stats: fold chunks, combine across b via PE selector --
                stats4 = persist.tile([P, 4], F32, tag="stats4")
                for i in range(4):
                    nc.vector.reduce_sum(out=stats4[:, i:i + 1],
                                         in_=sums[i][:, :], axis=AX.X)
                stp = psump.tile([CS, 4], F32, tag="stp")
                nc.tensor.matmul(stp[:, :], sel[:, :], stats4[:, :],
                                 start=True, stop=True, tile_position=(0, 0))
                statjs = persist.tile([CS, 4], F32, tag="statjs")
                nc.scalar.activation(out=statjs[:, :], in_=stp[:, :],
                                     func=AF.Copy)
                abj = persist.tile([CS, 4], F32, tag="abj")
                for i in range(2):
                    bn_coeffs(statjs[:, 2 * i:2 * i + 1],
                              statjs[:, 2 * i + 1:2 * i + 2], i,
                              abj[:, 2 * i:2 * i + 2], NSTAT)
                ab_d = dram.tile([CS, 4], F32, tag="ab_d")
                nc.sync.dma_start(out=ab_d[:, :], in_=abj[:, :])
                for i in range(2):
                    nc.sync.dma_start(
                        out=ab[i][:, :],
                        in_=bass.AP(tensor=ab_d.tensor,
                                    offset=ab_d.offset + 2 * i,
                                    ap=[[0, B], [4, CS], [1, 2]]))

                # ---- BN apply + relu into padded interiors ----
                for (ys, im, i) in ((y1s, i3d["1"], 0), (y2s, i3d["2"], 1)):
                    nc.scalar.activation(
                        out=im[:, 1:1 + HH, 1:1 + HH],
                        in_=ys[:, :].rearrange("p (r c) -> p r c", c=HH),
                        func=AF.Relu,
                        bias=ab[i][:, 1:2], scale=ab[i][:, 0:1])

            # ===== phase 2+3: attention chunks, partial conv, RS =====
            with ExitStack() as s2:
                fpool = s2.enter_context(tc.tile_pool(name="fbuf", bufs=2))
                cpool = s2.enter_context(tc.tile_pool(name="cbuf", bufs=2))
                tpool = s2.enter_context(tc.tile_pool(name="tbuf", bufs=2))
                gpool = s2.enter_context(tc.tile_pool(name="gbuf", bufs=2))
                orpool = s2.enter_context(tc.tile_pool(name="outr", bufs=2))
                psatt = s2.enter_context(
                    tc.tile_pool(name="psatt", bufs=2, space="PSUM"))
                psfc = s2.enter_context(
                    tc.tile_pool(name="psfc", bufs=2, space="PSUM"))
                evictf = s2.enter_context(tc.tile_pool(name="evictf", bufs=2))
                ybuff = s2.enter_context(tc.tile_pool(name="ybuff", bufs=1))
                yfs = ybuff.tile([P, L], BF, tag="yf")

                OUTr_of = {}

                def sub_widths(w):
                    return [(i * LCW, min(LCW, w - i * LCW))
                            for i in range((w + LCW - 1) // LCW)]

                def partial_piece(ci, b, h, off, ww):
                    c0, w = CHUNKS[ci]
                    OUTr = OUTr_of[ci]
                    pf = psfc.tile([P, LCW], F32, tag="pf", name="pf")
                    nc.tensor.matmul(
                        pf[:, :ww],
                        wfs[CS * b:CS * (b + 1), h * P:(h + 1) * P],
                        OUTr[CS * b:CS * (b + 1), off:off + ww],
                        start=True, stop=True,
                        tile_position=(CS * b, 0))
                    ev = evictf.tile([P, LCW], BF, tag="ev", name="ev")
                    nc.scalar.activation(out=ev[:, :ww], in_=pf[:, :ww],
                                         func=AF.Copy)
                    # partd addr(k=4h+a, p'=32b+j, l'); psum partition 32a+j
                    nc.sync.dma_start(
                        out=bass.AP(
                            tensor=partd[ci].tensor,
                            offset=(partd[ci].offset + h * 4 * P * w
                                    + b * CS * w + off),
                            ap=[[P * w, 4], [w, CS], [1, ww]]),
                        in_=ev[:, :ww])

                def emit_rs_cc(ci):
                    nc.gpsimd.collective_compute(
                        "ReduceScatter", mybir.AluOpType.add,
                        replica_groups=[list(range(NCORES))],
                        ins=[partd[ci][:, :].opt()],
                        outs=[recvd[ci][:, :].opt()])

                def emit_rs_post(ci):
                    c0, w = CHUNKS[ci]
                    nc.sync.dma_start(out=yfs[:, c0:c0 + w],
                                      in_=recvd[ci][:, :])
                    sq = evictf.tile([P, EXTMAX], BF, tag="sqf", name="sqf")
                    nc.scalar.activation(
                        out=sq[:, :w], in_=yfs[:, c0:c0 + w],
                        func=AF.Copy, accum_out=sumsf[0][:, ci:ci + 1])
                    nc.scalar.activation(
                        out=sq[:, :w], in_=yfs[:, c0:c0 + w],
                        func=AF.Square, accum_out=sumsf[1][:, ci:ci + 1])

                def pieces_of(ci):
                    return [(b, h, off, ww) for h in range(2)
                            for (off, ww) in sub_widths(CHUNKS[ci][1])
                            for b in range(B)]

                def emit_A(ci, t):
                    """one (chunk, shift-segment) iteration of the attention
                    stage -- bf16, spread across DVE/ACT/Pool."""
                    c0, w = CHUNKS[ci]
                    row0, nrows = c0 // HH, w // HH
                    di, dj = _shift(t)
                    r = (t + c0) % 9
                    s = (-(t + c0 + w)) % 9
                    ext = w + r + s
                    ng = ext // 9
                    F1c = fpool.tile([P, EXTMAX], BF, tag="F1c", name="F1c")
                    F2c = fpool.tile([P, EXTMAX], BF, tag="F2c", name="F2c")
                    for (Fc, im, iflat, eng) in (
                            (F1c, i3d["1"], img1, nc.vector),
                            (F2c, i3d["2"], img2, nc.gpsimd)):
                        eng.tensor_copy(
                            out=Fc[:, r:r + w].rearrange(
                                "p (i j) -> p i j", j=HH),
                            in_=im[:, 1 + di + row0:1 + di + row0 + nrows,
                                   1 + dj:1 + dj + HH])
                        if r:
                            if c0:
                                off = (di + row0) * PH + (65 - r + dj)
                            else:
                                pdi, pdj = _shift(t - 1)
                                off = (64 + pdi) * PH + (65 - r + pdj)
                            nc.vector.tensor_copy(out=Fc[:, 0:r],
                                                  in_=iflat[:, off:off + r])
                        if s:
                            if c0 + w < L:
                                off = (1 + di + row0 + nrows) * PH + 1 + dj
                            else:
                                ndi, ndj = _shift(t + 1)
                                off = (1 + ndi) * PH + (1 + ndj)
                            nc.vector.tensor_copy(
                                out=Fc[:, r + w:r + w + s],
                                in_=iflat[:, off:off + s])

                    F1g = F1c[:, :9 * ng].rearrange("p (g s) -> p g s", s=9)
                    F2g = F2c[:, :9 * ng].rearrange("p (g s) -> p g s", s=9)
                    # materialize packed center-broadcast arrays
                    C2p = cpool.tile([P, EXTMAX], BF, tag="C2p", name="C2p")
                    nc.scalar.copy(
                        out=C2p[:, :9 * ng].rearrange("p (g s) -> p g s", s=9),
                        in_=F2g[:, :, 4].unsqueeze(2).broadcast_to((P, ng, 9)))
                    C1p = cpool.tile([P, EXTMAX], BF, tag="C1p", name="C1p")
                    nc.gpsimd.tensor_copy(
                        out=C1p[:, :9 * ng].rearrange("p (g s) -> p g s", s=9),
                        in_=F1g[:, :, 4].unsqueeze(2).broadcast_to((P, ng, 9)))

                    t1 = tpool.tile([P, EXTMAX], BF, tag="t1", name="t1")
                    t2 = tpool.tile([P, EXTMAX], BF, tag="t2", name="t2")
                    lgb = tpool.tile([P, EXTMAX], BF, tag="lgb", name="lgb")
                    nc.vector.tensor_mul(t1[:, :ext], F1c[:, :ext],
                                         C2p[:, :ext])
                    nc.vector.tensor_mul(t2[:, :ext], F2c[:, :ext],
                                         C1p[:, :ext])
                    nc.vector.tensor_add(lgb[:, :ext], t1[:, :ext],
                                         t2[:, :ext])
                    eb = tpool.tile([P, EXTMAX], BF, tag="eb", name="eb")
                    nc.scalar.activation(out=eb[:, :ext], in_=lgb[:, :ext],
                                         func=AF.Exp, bias=expb_c[:, :])
                    e3 = eb[:, :9 * ng].rearrange("p (g s) -> p g s", s=9)
                    # group-sum tree: 4+4 packed, then fold, then center col 8
                    S4 = gpool.tile([P, 4 * NGMAX], BF, tag="S4", name="S4")
                    S4v = S4[:, :4 * ng].rearrange("p (g s) -> p g s", s=4)
                    nc.vector.tensor_add(S4v, e3[:, :, 0:4], e3[:, :, 4:8])
                    S2 = gpool.tile([P, 2 * NGMAX], BF, tag="S2", name="S2")
                    S2v = S2[:, :2 * ng].rearrange("p (g s) -> p g s", s=2)
                    nc.vector.tensor_add(S2v, S4v[:, :, 0:2], S4v[:, :, 2:4])
                    Sng = gpool.tile([P, NGMAX], F32, tag="Sng", name="Sng")
                    nc.vector.tensor_add(Sng[:, :ng], S2v[:, :, 0],
                                         S2v[:, :, 1])
                    nc.vector.tensor_add(Sng[:, :ng], Sng[:, :ng],
                                         e3[:, :, 8])
                    R = gpool.tile([P, NGMAX], F32, tag="R", name="R")
                    nc.vector.reciprocal(R[:, :ng], Sng[:, :ng])
                    Rp = cpool.tile([P, EXTMAX], BF, tag="Rp", name="Rp")
                    nc.scalar.copy(
                        out=Rp[:, :9 * ng].rearrange("p (g s) -> p g s", s=9),
                        in_=R[:, :ng].unsqueeze(2).broadcast_to((P, ng, 9)))
                    vb = tpool.tile([P, EXTMAX], BF, tag="vb", name="vb")
                    nc.vector.tensor_mul(vb[:, :ext], eb[:, :ext],
                                         Rp[:, :ext])
                    Pt = tpool.tile([P, EXTMAX], BF, tag="Pt", name="Pt")
                    nc.vector.tensor_mul(
                        Pt[:, :w].rearrange("p (i j) -> p i j", j=HH),
                        vb[:, r:r + w].rearrange("p (i j) -> p i j", j=HH),
                        i3d["3"][:, 1 + di + row0:1 + di + row0 + nrows,
                                 1 + dj:1 + dj + HH])
                    OUTP = OUTq_cur[0]
                    for (off, ww) in sub_widths(w):
                        nc.tensor.matmul(
                            OUTP[:, off:off + ww],
                            idm[:, :],
                            Pt[:, off:off + ww],
                            start=(t == 0), stop=(t == 8),
                            tile_position=(0, 0), skip_group_check=True)

                OUTq_cur = [None]
                for ci, (c0, w) in enumerate(CHUNKS):
                    prev = pieces_of(ci - 1) if ci > 0 else []
                    npc = len(prev)
                    OUTP = psatt.tile([P, 1536], F32, tag="OUTP",
                                      name=f"OUTP{ci}")
                    OUTq_cur[0] = OUTP
                    for t in range(9):
                        # interleave previous chunk's partial final conv
                        if ci > 0 and t < 8:
                            lo = (npc * t) // 8
                            hi = (npc * (t + 1)) // 8
                            for pc in prev[lo:hi]:
                                partial_piece(ci - 1, *pc)
                        elif ci > 0 and t == 8:
                            emit_rs_cc(ci - 1)
                        if ci > 1 and t == 2:
                            emit_rs_post(ci - 2)
                        emit_A(ci, t)

                    # round pre chunk to bf16 for the partial conv PE pass
                    OUTr = orpool.tile([P, max(w for _, w in CHUNKS)], BF,
                                       tag="OUTr", name="OUTr")
                    nc.scalar.copy(out=OUTr[:, :w], in_=OUTP[:, :w])
                    OUTr_of[ci] = OUTr

                # drain the last chunk's partial conv + RS
                for pc in pieces_of(NCH - 1):
                    partial_piece(NCH - 1, *pc)
                emit_rs_cc(NCH - 1)
                emit_rs_post(NCH - 2)
                emit_rs_post(NCH - 1)

                # ---- final BN stats + apply + output ----
                stats2 = persist.tile([P, 2], F32, tag="stats2")
                for i in range(2):
                    nc.vector.reduce_sum(out=stats2[:, i:i + 1],
                                         in_=sumsf[i][:, :], axis=AX.X)
                stpf_t = psfc.tile([P, LCW], F32, tag="pf", name="stpf")
                stpf = stpf_t[0:CS, 0:2]
                nc.tensor.matmul(stpf, sel[:, :], stats2[:, :],
                                 start=True, stop=True, tile_position=(0, 0))
                statjsf = persist.tile([CS, 2], F32, tag="statjsf")
                nc.scalar.activation(out=statjsf[:, :], in_=stpf,
                                     func=AF.Copy)
                abjf = persist.tile([CS, 2], F32, tag="abjf")
                bn_coeffs(statjsf[:, 0:1], statjsf[:, 1:2], 2,
                          abjf[:, 0:2], NSTAT)
                abpf_t = psfc.tile([P, LCW], F32, tag="pf", name="abpf")
                abpf = abpf_t[:, 0:2]
                nc.tensor.matmul(abpf, selT[:, :], abjf[:, :],
                                 start=True, stop=True, tile_position=(0, 0))
                nc.scalar.activation(out=ab[2][:, :], in_=abpf,
                                     func=AF.Copy)
                fout = ybuff.tile([P, L], BF, tag="fout")
                nc.scalar.activation(out=fout[:, :], in_=yfs[:, :],
                                     func=AF.Relu,
                                     bias=ab[2][:, 1:2], scale=ab[2][:, 0:1])
                nc.sync.dma_start(out=out_t[:, :], in_=fout[:, :])

    nc.finalize()
    return nc


_NC_CACHE = None


def _get_nc():
    global _NC_CACHE
    if _NC_CACHE is None:
        _NC_CACHE = build_program()
    return _NC_CACHE


def make_in_maps(inputs):
    x = np.asarray(inputs["x"], np.float32).reshape(B, C, L)
    xbf = np.ascontiguousarray(x.astype(ml_dtypes.bfloat16))
    w1 = np.asarray(inputs["w1"], np.float32)
    w2 = np.asarray(inputs["w2"], np.float32)
    wf = np.asarray(inputs["wf"], np.float32)
    maps = []
    for k in range(NCORES):
        sl = slice(k * CS, (k + 1) * CS)
        # padded conv weights: block kb = h*4+b has [:, 32b:32b+32] =
        # w[sl].T[128h:128(h+1), :]
        w1p = np.zeros((P, 8 * P), np.float32)
        w2p = np.zeros((P, 8 * P), np.float32)
        for h in range(2):
            for b in range(B):
                kb = h * B + b
                w1p[:, P * kb + CS * b:P * kb + CS * (b + 1)] = \
                    w1[sl, h * P:(h + 1) * P].T
                w2p[:, P * kb + CS * b:P * kb + CS * (b + 1)] = \
                    w2[sl, h * P:(h + 1) * P].T
        # wf for the partial final conv: [32, 128] block replicated on all
        # four 32-partition bands (lhsT rows must sit on the rhs partitions)
        wflp = np.zeros((P, 2 * P), np.float32)
        for h in range(2):
            blk = wf[h * P:(h + 1) * P, sl].T          # [32, 128]
            wflp[:, h * P:(h + 1) * P] = np.tile(blk, (B, 1))
        m = {
            "xb": xbf,
            "xsb": np.ascontiguousarray(
                x[:, sl, :].reshape(P, L).astype(ml_dtypes.bfloat16)),
            "w1p": np.ascontiguousarray(w1p.astype(ml_dtypes.bfloat16)),
            "w2p": np.ascontiguousarray(w2p.astype(ml_dtypes.bfloat16)),
            "wfl": np.ascontiguousarray(wflp.astype(ml_dtypes.bfloat16)),
            "bnp": np.ascontiguousarray(np.stack([
                np.asarray(inputs["g1"], np.float32)[sl],
                np.asarray(inputs["be1"], np.float32)[sl],
                np.asarray(inputs["g2"], np.float32)[sl],
                np.asarray(inputs["be2"], np.float32)[sl],
                np.asarray(inputs["gf"], np.float32)[sl],
                np.asarray(inputs["bef"], np.float32)[sl],
            ])),
        }
        maps.append(m)
    return maps


def run(inputs, trace=False):
    nc = _get_nc()
    in_maps = make_in_maps(inputs)
    res = run_bass_kernel_spmd(nc, in_maps, core_ids=list(range(NCORES)),
                               trace=trace)
    full = np.empty((B, C, HH, HH), np.float32)
    for k in range(NCORES):
        full[:, k * CS:(k + 1) * CS] = np.asarray(
            res.results[k]["out"], dtype=np.float32).reshape(B, CS, HH, HH)
    return full, res


def kernel(**inputs) -> np.ndarray:
    out, _ = run(inputs, trace=False)
    return out
